# revision 23
# baseline (speedup 1.0000x reference)
"""Trainium2 Bass kernel for AssociativeIncrementalAttention.

Multi-head attention (B=2, S=2048, D=512, H=8, HD=64) with additive
[B,S,S] bias tensors, a concept-equality bias, and key-padding mask.

Sharding: 8 cores, fully data-parallel (no collectives).
  core c -> batch b = c//4, query rows q0 = (c%4)*512 .. q0+512.

v2 design notes (engine-work reduction vs v1):
  - All large DRAM inputs are host-cast to bf16 (halves HBM traffic and
    the startup DMA head; matmuls ran in bf16 anyway).
  - Concept bias (same concept id, both valid, off-diagonal) is computed
    on the PE as a one-hot inner product: cs = onehotK^T @ (0.5*onehotQ)
    over the 64 concept ids, 16 tiny matmuls; replaces ~35us of DVE
    compare work. Diagonal exclusion multiplies cs by (qpos != kpos) on
    the 4 key chunks that contain the diagonal (exact, incl. cid<0).
  - bk is dropped exactly: softmax over k is invariant to the q.bk and
    bq.bk terms of (q+bq).(k+bk); only bq.k survives, so bq stays on Q
    and K needs no bias.  bv/bo ride the output epilogue (cvec) since
    softmax rows sum to 1.
  - Scalar (Act) engine does exp only (plus tiny psq-identity); PSUM->
    SBUF copies are split between DVE/Act/GpSimd to balance busy time.
  - Softmax normalization deferred past attn@V (rowsum rides a ones
    column in V); applied as recip + partition_broadcast + multiply,
    interleaved into the next head's groups.
  - attention_mask / key_padding_mask are all-zero in the target inputs;
    the fast variant skips them (host-checked), a generic variant with
    both is compiled lazily if they are ever nonzero.

Self-contained: hardcodes shapes; host-side prep is layout-only
(slices/transposes) plus dtype casts and tiny metadata encodings
(concept ids -> bf16 sentinel values, position iotas).
"""

import sys

if "/opt/trn_rl_repo" not in sys.path:
    sys.path.insert(0, "/opt/trn_rl_repo")

import numpy as np
import ml_dtypes

import concourse.bass as bass
import concourse.tile as tile
from concourse import bacc, mybir
from concourse import bass_utils

B, S, D, H = 2, 2048, 512, 8
HD = D // H  # 64
N_CORES = 8
QS = 512          # query rows per core
QT = QS // 128    # 4 query tiles per core
DC = D // 128     # 4 contraction chunks
SC512 = S // 512  # 4
SC128 = S // 128  # 16
NC = 64           # number of concept ids
F32 = mybir.dt.float32
F16 = mybir.dt.float16
BF16 = mybir.dt.bfloat16

BF = ml_dtypes.bfloat16

_COMPILED = {}


def _build(with_masks: bool):
    nc = bacc.Bacc("TRN2", target_bir_lowering=False, debug=False,
                   num_devices=N_CORES)

    def din(name, shape, dt=BF16):
        return nc.dram_tensor(name, shape, dt, kind="ExternalInput").ap()

    xT = din("xT", [D, S])            # x[b].T
    xqT = din("xqT", [D, QS])         # x[b, q0:q0+QS].T
    wqT = din("wqT", [D, D])
    wkT = din("wkT", [D, D])
    wvT = din("wvT", [D, D])
    woT = din("woT", [D, D])
    bq = din("bq", [D], F32)
    bv = din("bv", [D], F32)
    bo = din("bo", [D], F32)
    ipaT = din("ipaT", [S, QS])
    ascT = din("ascT", [S, QS])
    if with_masks:
        mskT = din("mskT", [S, QS])
        kpmT = din("kpmT", [128, SC128], F32)
    cidq = din("cidq", [QS])          # concept ids (neg -> -1 sentinel)
    cidk = din("cidk", [S])           # concept ids (neg -> -2 sentinel)
    iota64 = din("iota64", [NC], F32)
    qpos = din("qpos", [QS], F16)
    kposT = din("kposT", [128, SC128], F32)
    out = nc.dram_tensor("out", [QS, D], BF16, kind="ExternalOutput").ap()

    AL = mybir.AluOpType
    AF = mybir.ActivationFunctionType

    def bcast_ap(src, parts):
        # partition-broadcast read: [[0,parts]] + original free dims
        return bass.AP(tensor=src.tensor, offset=src.offset,
                       ap=[[0, parts]] + list(src.ap))

    # which kc chunks contain diagonal cells depends on q0 (per-core), but
    # the kernel is compiled once for all cores. The diagonal columns are
    # found at runtime by comparing qpos/kpos tiles, so we simply apply
    # the (qpos != kpos) multiply on ALL chunks' pairs... that would cost
    # 16 extra ops; instead exploit that each core's diagonal lies in the
    # 4 chunks q0/128..q0/128+3 -- but q0 is per-core. Trick: qpos/kpos
    # comparison is data-driven, so apply the diag-multiply on all 8
    # pairs only if needed. Cheaper: host passes kpos shifted so that the
    # compare is false outside the diag chunks anyway; the multiply by
    # e in {0,1} is exact everywhere, so applying it everywhere is always
    # correct. We bound cost by applying per-pair (8 pairs) with a
    # [128,1]-scalar compare per chunk (16 ts + 16 tt).
    # -> final choice: apply on all chunks (uniform, data-driven, safe).

    with tile.TileContext(nc) as tc:
        with (
            tc.tile_pool(name="persist", bufs=1) as P,
            tc.tile_pool(name="ipain", bufs=2) as IPA,
            tc.tile_pool(name="ascin", bufs=2) as ASC,
            tc.tile_pool(name="xfall", bufs=3) as XF,
            tc.tile_pool(name="ework", bufs=3) as EW,
            tc.tile_pool(name="praww", bufs=10) as PRW,
            tc.tile_pool(name="pfw", bufs=8) as PW,
            tc.tile_pool(name="rswork", bufs=2) as RW,
            tc.tile_pool(name="osb", bufs=2) as OS,
            tc.tile_pool(name="spp", bufs=3, space="PSUM") as SPP,
            tc.tile_pool(name="ctxp", bufs=2, space="PSUM") as CP,
        ):
            # ---- persistent tiles ----
            kT_sb = P.tile([128, DC, S], BF16, tag="kT")
            qT_sb = P.tile([128, DC, QS], BF16, tag="qT")
            vplus = P.tile([128, SC128, 8 * 65], BF16, tag="vplus")
            vp4 = vplus.rearrange("p s (h c) -> p s h c", c=65)
            ebT = P.tile([128, SC128, QS], BF16, tag="ebT")
            ctxT_sb = P.tile([128, DC, QS], BF16, tag="ctxT")

            # ---- small input loads (sync queue) ----
            iota_col = P.tile([NC, 1], F32, tag="iota_col")
            nc.sync.dma_start(out=iota_col,
                              in_=iota64.rearrange("(p a) -> p a", a=1))
            cidkb = P.tile([NC, S], BF16, tag="cidkb")
            nc.sync.dma_start(out=cidkb, in_=bcast_ap(cidk, NC))
            cidqb = P.tile([NC, QS], BF16, tag="cidqb")
            nc.sync.dma_start(out=cidqb, in_=bcast_ap(cidq, NC))
            qposb = P.tile([128, QS], F16, tag="qposb")
            nc.sync.dma_start(out=qposb, in_=bcast_ap(qpos, 128))
            kposT_sb = P.tile([128, SC128], F32, tag="kposT")
            nc.sync.dma_start(out=kposT_sb, in_=kposT)
            b_sb = {}
            for nm, ap_ in (("bq", bq), ("bv", bv)):
                b_sb[nm] = P.tile([128, DC], F32, tag=nm, name=nm)
                nc.sync.dma_start(out=b_sb[nm],
                                  in_=ap_.rearrange("(c p) -> p c", p=128))
            bo_row = P.tile([1, D], F32, tag="bo_row")
            nc.sync.dma_start(out=bo_row, in_=bo.rearrange("(a s) -> a s", a=1))
            if with_masks:
                kpmT_sb = P.tile([128, SC128], F32, tag="kpmT")
                nc.sync.dma_start(out=kpmT_sb, in_=kpmT)

            # ---- big input loads (gpsimd queue; issue order ~ arrival) ----
            w_sb = {}

            def load_w(nm, ap_):
                w_sb[nm] = P.tile([128, DC, D], BF16, tag=nm, name=nm)
                nc.gpsimd.dma_start(
                    out=w_sb[nm], in_=ap_.rearrange("(c p) s -> p c s", p=128))

            load_w("wq", wqT)
            xqT_sb = P.tile([128, DC, QS], BF16, tag="xqT")
            nc.gpsimd.dma_start(
                out=xqT_sb, in_=xqT.rearrange("(c p) s -> p c s", p=128))
            load_w("wk", wkT)
            load_w("wv", wvT)
            xT_sb = P.tile([128, DC, S], BF16, tag="xT")

            def load_x_sc(sc):
                nc.gpsimd.dma_start(
                    out=xT_sb[:, :, sc * 512:(sc + 1) * 512],
                    in_=xT.rearrange("(c p) s -> p c s", p=128)[
                        :, :, sc * 512:(sc + 1) * 512])

            bias_in = {"ipa": [None] * 4, "asc": [None] * 4,
                       "msk": [None] * 4}

            def load_bias_g4(g4):
                specs = [("ipa", ipaT, IPA), ("asc", ascT, ASC)]
                if with_masks:
                    specs.append(("msk", mskT, EW))
                for nm, ap_, pool in specs:
                    t = pool.tile([128, 4, QS], BF16, tag="b" + nm, name=nm)
                    nc.gpsimd.dma_start(
                        out=t,
                        in_=ap_[g4 * 512:(g4 + 1) * 512, :].rearrange(
                            "(c p) s -> p c s", p=128))
                    bias_in[nm][g4] = t

            # biases first among the big streams: the eb pairs gate the
            # pf/attnV chain for EVERY head, while xT only gates the
            # progressive kT/v production consumed one block per h0 group
            load_bias_g4(0)
            load_bias_g4(1)
            load_x_sc(0)
            load_bias_g4(2)
            load_bias_g4(3)
            load_x_sc(1)
            load_x_sc(2)
            load_x_sc(3)
            load_w("wo", woT)
            wo_sb = w_sb["wo"]

            bv_bf = P.tile([128, DC], BF16, tag="bv_bf")
            nc.vector.tensor_copy(bv_bf, b_sb["bv"])
            cvec = P.tile([1, D], BF16, tag="cvec")
            ones_row = P.tile([1, 128], BF16, tag="ones_row")
            nc.vector.memset(ones_row, 1.0)
            nc.vector.memset(vp4[:, :, :, 64:65], 1.0)

            # ---- one-hot concept encodings (DVE, tiny) ----
            ohk = P.tile([NC, S], BF16, tag="ohk")
            nc.vector.tensor_scalar(
                out=ohk, in0=cidkb, scalar1=iota_col, scalar2=None,
                op0=AL.is_equal)
            ohq = P.tile([NC, QS], BF16, tag="ohq")
            nc.vector.tensor_scalar(
                out=ohq, in0=cidqb, scalar1=iota_col, scalar2=0.5,
                op0=AL.is_equal, op1=AL.mult)

            # ---- concept-score matmuls (PSUM tiles consumed promptly by the
            # combine now that biases are front-loaded in the DMA order) ----
            cs_tiles = []
            for p in range(8):
                cs = SPP.tile([128, 2, 512], F32, tag="sp", name="cs")
                for j in range(2):
                    kc = p * 2 + j
                    nc.tensor.matmul(
                        cs[:, j, :],
                        lhsT=ohk[:, kc * 128:(kc + 1) * 128],
                        rhs=ohq,
                        start=True, stop=True)
                cs_tiles.append(cs)

            # ---- Q projection (+bq via Act identity) ----
            for ocp in range(2):
                ps = SPP.tile([128, 2, 512], F32, tag="sp", name="psq")
                for i in range(2):
                    oc = ocp * 2 + i
                    for dc in range(DC):
                        nc.tensor.matmul(
                            ps[:, i, :],
                            lhsT=w_sb["wq"][:, dc, oc * 128:(oc + 1) * 128],
                            rhs=xqT_sb[:, dc, :],
                            start=(dc == 0), stop=(dc == DC - 1))
                for i in range(2):
                    oc = ocp * 2 + i
                    nc.scalar.activation(
                        out=qT_sb[:, oc, :], in_=ps[:, i, :],
                        func=AF.Identity, bias=b_sb["bq"][:, oc:oc + 1])

            # ---- K projection (sc-major so scores can start early) ----
            for sc in range(SC512):
                for ocp in range(2):
                    ps = SPP.tile([128, 2, 512], F32, tag="sp", name="psk")
                    for i in range(2):
                        oc = ocp * 2 + i
                        for dc in range(DC):
                            nc.tensor.matmul(
                                ps[:, i, :],
                                lhsT=w_sb["wk"][:, dc,
                                                oc * 128:(oc + 1) * 128],
                                rhs=xT_sb[:, dc, sc * 512:(sc + 1) * 512],
                                start=(dc == 0), stop=(dc == DC - 1))
                    dst = kT_sb[:, ocp * 2:(ocp + 1) * 2,
                                sc * 512:(sc + 1) * 512]
                    if sc < 2:
                        nc.vector.tensor_copy(dst, ps)
                    else:
                        nc.scalar.copy(out=dst, in_=ps)

            # ---- V projection -> vplus (copies on gpsimd) ----
            for scp in range(8):
                ps = SPP.tile([128, 2, 512], F32, tag="sp", name="psv")
                for i in range(2):
                    sc = scp * 2 + i
                    for dc in range(DC):
                        nc.tensor.matmul(
                            ps[:, i, :],
                            lhsT=xT_sb[:, dc, sc * 128:(sc + 1) * 128],
                            rhs=w_sb["wv"][:, dc, :],
                            start=(dc == 0), stop=(dc == DC - 1))
                vdst = vp4[:, scp * 2:(scp + 1) * 2, :, 0:64]
                vsrc = ps.rearrange("p i (h c) -> p i h c", c=64)
                if scp % 2 == 0:
                    nc.vector.tensor_copy(vdst, vsrc)
                else:
                    nc.scalar.copy(out=vdst, in_=vsrc)

            # ---- combined bias -> exp, in [k, q] layout, per kc pair ----
            # xfall = cs*(qpos!=kpos) + ipa + asc (+ msk + kpm); eb = exp().
            def combine_pair(p):
                g4 = p // 2
                ipa_t = bias_in["ipa"][g4]
                asc_t = bias_in["asc"][g4]
                jj = (p % 2) * 2
                cs = cs_tiles[p]
                xf = XF.tile([128, 2, QS], BF16, tag="xf", name="xf")
                for j in range(2):
                    kc = p * 2 + j
                    # xf = (qpos != kpos) * cs  -- one fused op per chunk,
                    # reading cs straight from PSUM (frees the ring slot)
                    nc.vector.scalar_tensor_tensor(
                        out=xf[:, j, :], in0=qposb,
                        scalar=kposT_sb[:, kc:kc + 1], in1=cs[:, j, :],
                        op0=AL.not_equal, op1=AL.mult)
                t1 = XF.tile([128, 2, QS], BF16, tag="t1", name="t1")
                nc.vector.tensor_tensor(
                    out=t1, in0=xf, in1=ipa_t[:, jj:jj + 2, :], op=AL.add)
                dst = XF.tile([128, 2, QS], BF16, tag="t2", name="t2")
                nc.vector.tensor_tensor(
                    out=dst, in0=t1, in1=asc_t[:, jj:jj + 2, :], op=AL.add)
                if with_masks:
                    dst2 = XF.tile([128, 2, QS], BF16, tag="t3", name="t3")
                    nc.vector.tensor_tensor(
                        out=dst2, in0=dst, in1=bias_in["msk"][g4][:, jj:jj + 2, :],
                        op=AL.add)
                    kcol = EW.tile([128, 2, QS], BF16, tag="kp", name="kp")
                    for j in range(2):
                        kc = p * 2 + j
                        nc.vector.tensor_scalar(
                            out=kcol[:, j, :], in0=dst2[:, j, :],
                            scalar1=kpmT_sb[:, kc:kc + 1], scalar2=None,
                            op0=AL.add)
                    dst = kcol
                # exp on Act
                nc.scalar.activation(
                    out=ebT[:, p * 2:(p + 1) * 2, :], in_=dst, func=AF.Exp)

            for p in range(8):
                combine_pair(p)

            # ---- attention main loop ----
            norm_state = {}

            def norm_step(step, hprev, cps_prev):
                ocp_, rbp = hprev // 2, (hprev % 2) * 64
                if step == 0:
                    rs_row = RW.tile([1, QS], F32, tag="rs_row",
                                     name="rs_row")
                    nc.vector.tensor_copy(rs_row, cps_prev[64:65, :])
                    rr = RW.tile([1, QS], F32, tag="rr", name="rr")
                    nc.vector.reciprocal_approx_fast(rr, rs_row)
                    norm_state["rr"] = rr
                elif step == 1:
                    rrb = RW.tile([64, QS], F32, tag="rrb", name="rrb")
                    nc.gpsimd.partition_broadcast(rrb, norm_state["rr"])
                    norm_state["rrb"] = rrb
                else:
                    nc.vector.tensor_tensor(
                        out=ctxT_sb[rbp:rbp + 64, ocp_, :],
                        in0=cps_prev[0:64, :], in1=norm_state["rrb"],
                        op=AL.mult)

            # attnV emission lags the scores/exp stream by ATTNV_LAG groups
            # so the in-order PE queue never stalls waiting for praw/pf of
            # the group it just produced (software pipelining).
            ATTNV_LAG = 6

            def emit_attnv(cps_t, h, g, pf):
                for j in range(2):
                    kc = g * 2 + j
                    nc.tensor.matmul(
                        cps_t,
                        lhsT=vplus[:, kc, h * 65:(h + 1) * 65],
                        rhs=pf[:, j, :],
                        start=(kc == 0), stop=(kc == SC128 - 1))

            pending = None
            attnv_q = []
            for h in range(H):
                oc, rb = h // 2, (h % 2) * 64
                cps_h = CP.tile([65, QS], F32, tag="ctx", name="ctx")
                for g in range(8):
                    ps = SPP.tile([128, 2, 512], F32, tag="sp", name="pss")
                    for j in range(2):
                        kc = g * 2 + j
                        nc.tensor.matmul(
                            ps[:, j, :],
                            lhsT=kT_sb[rb:rb + 64, oc,
                                       kc * 128:(kc + 1) * 128],
                            rhs=qT_sb[rb:rb + 64, oc, :],
                            start=True, stop=True)
                    praw = PRW.tile([128, 2, 512], BF16, tag="praw",
                                    name="praw")
                    nc.scalar.activation(out=praw, in_=ps, func=AF.Exp,
                                         scale=0.125)
                    pf = PW.tile([128, 2, 512], BF16, tag="pf", name="pf")
                    nc.vector.tensor_tensor(
                        out=pf, in0=praw,
                        in1=ebT[:, g * 2:(g + 1) * 2, :], op=AL.mult)
                    attnv_q.append((cps_h, h, g, pf))
                    if len(attnv_q) > ATTNV_LAG:
                        emit_attnv(*attnv_q.pop(0))
                    if pending is not None and 5 <= g <= 7:
                        norm_step(g - 5, *pending)
                        if g == 7:
                            pending = None
                pending = (h, cps_h)
            while attnv_q:
                emit_attnv(*attnv_q.pop(0))

            # cvec = Wo @ bv + bo  (rank-1 epilogue row)
            cps = SPP.tile([128, 2, 512], F32, tag="sp", name="cps")
            for dc in range(DC):
                nc.tensor.matmul(cps[0:1, 0, :], lhsT=bv_bf[:, dc:dc + 1],
                                 rhs=wo_sb[:, dc, :],
                                 start=(dc == 0), stop=(dc == DC - 1))
            nc.vector.tensor_tensor(out=cvec, in0=cps[0:1, 0, :], in1=bo_row,
                                    op=AL.add)

            for step in range(3):
                norm_step(step, *pending)

            # ---- output projection ----
            for m in range(QT):
                pom = SPP.tile([128, 2, 512], F32, tag="sp", name="pom")
                for dc in range(DC):
                    nc.tensor.matmul(
                        pom[:, 0, :],
                        lhsT=ctxT_sb[:, dc, m * 128:(m + 1) * 128],
                        rhs=wo_sb[:, dc, :],
                        start=(dc == 0), stop=False)
                nc.tensor.matmul(pom[:, 0, :], lhsT=ones_row, rhs=cvec,
                                 start=False, stop=True)
                o_t = OS.tile([128, 512], BF16, tag="o", name="o_t")
                nc.vector.tensor_copy(o_t, pom[:, 0, :])
                nc.sync.dma_start(out=out[m * 128:(m + 1) * 128, :],
                                  in_=o_t)

    nc.compile()
    return nc


def _prep_in_maps(inputs, with_masks):
    x = np.asarray(inputs["x"], np.float32)
    ipa = np.asarray(inputs["ipa_affinity_bias"], np.float32)
    asc = np.asarray(inputs["assoc_bias"], np.float32)
    msk = np.asarray(inputs["attention_mask"], np.float32)
    cid = np.asarray(inputs["concept_ids"])
    kpm = np.asarray(inputs["key_padding_mask"])

    wT = {nm: np.ascontiguousarray(
        np.asarray(inputs[nm], np.float32).T.astype(BF))
        for nm in ("Wq", "Wk", "Wv", "Wo")}
    bias = {nm: np.asarray(inputs[nm], np.float32)
            for nm in ("bq", "bv", "bo")}

    xT = [np.ascontiguousarray(x[b].T.astype(BF)) for b in range(B)]
    ipaT = [[np.ascontiguousarray(
        ipa[b, q0:q0 + QS].T.astype(BF)) for q0 in range(0, S, QS)]
        for b in range(B)]
    ascT = [[np.ascontiguousarray(
        asc[b, q0:q0 + QS].T.astype(BF)) for q0 in range(0, S, QS)]
        for b in range(B)]
    cidq_f = np.where(cid >= 0, cid, -1).astype(BF)
    cidk_f = np.where(cid >= 0, cid, -2).astype(BF)
    kpm_add = np.where(kpm, np.float32(-1e30), np.float32(0.0))
    kpos = np.arange(S, dtype=np.float32)
    iota64 = np.arange(NC, dtype=np.float32)

    in_maps = []
    for c in range(N_CORES):
        b, qi = c // 4, c % 4
        q0 = qi * QS
        m = {
            "xT": xT[b],
            "xqT": np.ascontiguousarray(xT[b][:, q0:q0 + QS]),
            "wqT": wT["Wq"], "wkT": wT["Wk"],
            "wvT": wT["Wv"], "woT": wT["Wo"],
            "bq": bias["bq"], "bv": bias["bv"], "bo": bias["bo"],
            "ipaT": ipaT[b][qi],
            "ascT": ascT[b][qi],
            "cidq": np.ascontiguousarray(cidq_f[b, q0:q0 + QS]),
            "cidk": np.ascontiguousarray(cidk_f[b]),
            "iota64": iota64,
            "qpos": (q0 + np.arange(QS)).astype(np.float16),
            "kposT": np.ascontiguousarray(kpos.reshape(SC128, 128).T),
        }
        if with_masks:
            m["mskT"] = np.ascontiguousarray(
                msk[q0:q0 + QS].T.astype(BF))
            m["kpmT"] = np.ascontiguousarray(
                kpm_add[b].reshape(SC128, 128).T)
        in_maps.append(m)
    return in_maps


def run(inputs, trace=False):
    msk = np.asarray(inputs["attention_mask"])
    kpm = np.asarray(inputs["key_padding_mask"])
    with_masks = bool(msk.any() or kpm.any())
    if with_masks not in _COMPILED:
        _COMPILED[with_masks] = _build(with_masks)
    nc = _COMPILED[with_masks]
    in_maps = _prep_in_maps(inputs, with_masks)
    kw = {}
    if trace:
        kw = dict(trace=True, trace_cores=list(range(N_CORES)))
    res = bass_utils.run_bass_kernel_spmd(
        nc, in_maps, core_ids=list(range(N_CORES)), **kw)
    out = np.empty((B, S, D), np.float32)
    for c in range(N_CORES):
        b, q0 = c // 4, (c % 4) * QS
        out[b, q0:q0 + QS] = np.asarray(res.results[c]["out"],
                                        dtype=np.float32)
    return out, res


def kernel(**inputs) -> np.ndarray:
    out, _ = run(inputs)
    return out


# revision 24
# speedup vs baseline: 1.0435x; 1.0435x over previous
"""Trainium2 Bass kernel for AssociativeIncrementalAttention.

Multi-head attention (B=2, S=2048, D=512, H=8, HD=64) with additive
[B,S,S] bias tensors, a concept-equality bias, and key-padding mask.

Sharding: 8 cores, fully data-parallel (no collectives).
  core c -> batch b = c//4, query rows q0 = (c%4)*512 .. q0+512.

v2 design notes (engine-work reduction vs v1):
  - All large DRAM inputs are host-cast to bf16 (halves HBM traffic and
    the startup DMA head; matmuls ran in bf16 anyway).
  - Concept bias (same concept id, both valid, off-diagonal) is computed
    on the PE as a one-hot inner product: cs = onehotK^T @ (0.5*onehotQ)
    over the 64 concept ids, 16 tiny matmuls; replaces ~35us of DVE
    compare work. Diagonal exclusion multiplies cs by (qpos != kpos) on
    the 4 key chunks that contain the diagonal (exact, incl. cid<0).
  - bk is dropped exactly: softmax over k is invariant to the q.bk and
    bq.bk terms of (q+bq).(k+bk); only bq.k survives, so bq stays on Q
    and K needs no bias.  bv/bo ride the output epilogue (cvec) since
    softmax rows sum to 1.
  - Scalar (Act) engine does exp only (plus tiny psq-identity); PSUM->
    SBUF copies are split between DVE/Act/GpSimd to balance busy time.
  - Softmax normalization deferred past attn@V (rowsum rides a ones
    column in V); applied as recip + partition_broadcast + multiply,
    interleaved into the next head's groups.
  - attention_mask / key_padding_mask are all-zero in the target inputs;
    the fast variant skips them (host-checked), a generic variant with
    both is compiled lazily if they are ever nonzero.

Self-contained: hardcodes shapes; host-side prep is layout-only
(slices/transposes) plus dtype casts and tiny metadata encodings
(concept ids -> bf16 sentinel values, position iotas).
"""

import sys

if "/opt/trn_rl_repo" not in sys.path:
    sys.path.insert(0, "/opt/trn_rl_repo")

import numpy as np
import ml_dtypes

import concourse.bass as bass
import concourse.tile as tile
from concourse import bacc, mybir
from concourse import bass_utils

B, S, D, H = 2, 2048, 512, 8
HD = D // H  # 64
N_CORES = 8
QS = 512          # query rows per core
QT = QS // 128    # 4 query tiles per core
DC = D // 128     # 4 contraction chunks
SC512 = S // 512  # 4
SC128 = S // 128  # 16
NC = 64           # number of concept ids
F32 = mybir.dt.float32
F16 = mybir.dt.float16
BF16 = mybir.dt.bfloat16

BF = ml_dtypes.bfloat16

_COMPILED = {}


def _build(with_masks: bool):
    nc = bacc.Bacc("TRN2", target_bir_lowering=False, debug=False,
                   num_devices=N_CORES)

    def din(name, shape, dt=BF16):
        return nc.dram_tensor(name, shape, dt, kind="ExternalInput").ap()

    xT = din("xT", [D, S])            # x[b].T
    xqT = din("xqT", [D, QS])         # x[b, q0:q0+QS].T
    wqT = din("wqT", [D, D])
    wkT = din("wkT", [D, D])
    wvT = din("wvT", [D, D])
    woT = din("woT", [D, D])
    bq = din("bq", [D], F32)
    bv = din("bv", [D], F32)
    bo = din("bo", [D], F32)
    ipaT = din("ipaT", [S, QS])
    ascT = din("ascT", [S, QS])
    if with_masks:
        mskT = din("mskT", [S, QS])
        kpmT = din("kpmT", [128, SC128], F32)
    cidq = din("cidq", [QS])          # concept ids (neg -> -1 sentinel)
    cidk = din("cidk", [S])           # concept ids (neg -> -2 sentinel)
    iota64 = din("iota64", [NC], F32)
    qpos = din("qpos", [QS], F16)
    kposT = din("kposT", [128, SC128], F32)
    out = nc.dram_tensor("out", [QS, D], BF16, kind="ExternalOutput").ap()

    AL = mybir.AluOpType
    AF = mybir.ActivationFunctionType

    def bcast_ap(src, parts):
        # partition-broadcast read: [[0,parts]] + original free dims
        return bass.AP(tensor=src.tensor, offset=src.offset,
                       ap=[[0, parts]] + list(src.ap))

    # which kc chunks contain diagonal cells depends on q0 (per-core), but
    # the kernel is compiled once for all cores. The diagonal columns are
    # found at runtime by comparing qpos/kpos tiles, so we simply apply
    # the (qpos != kpos) multiply on ALL chunks' pairs... that would cost
    # 16 extra ops; instead exploit that each core's diagonal lies in the
    # 4 chunks q0/128..q0/128+3 -- but q0 is per-core. Trick: qpos/kpos
    # comparison is data-driven, so apply the diag-multiply on all 8
    # pairs only if needed. Cheaper: host passes kpos shifted so that the
    # compare is false outside the diag chunks anyway; the multiply by
    # e in {0,1} is exact everywhere, so applying it everywhere is always
    # correct. We bound cost by applying per-pair (8 pairs) with a
    # [128,1]-scalar compare per chunk (16 ts + 16 tt).
    # -> final choice: apply on all chunks (uniform, data-driven, safe).

    with tile.TileContext(nc) as tc:
        with (
            tc.tile_pool(name="persist", bufs=1) as P,
            tc.tile_pool(name="ipain", bufs=2) as IPA,
            tc.tile_pool(name="ascin", bufs=2) as ASC,
            tc.tile_pool(name="xfall", bufs=3) as XF,
            tc.tile_pool(name="ework", bufs=3) as EW,
            tc.tile_pool(name="praww", bufs=10) as PRW,
            tc.tile_pool(name="pfw", bufs=8) as PW,
            tc.tile_pool(name="rswork", bufs=2) as RW,
            tc.tile_pool(name="osb", bufs=2) as OS,
            tc.tile_pool(name="spp", bufs=3, space="PSUM") as SPP,
            tc.tile_pool(name="ctxp", bufs=2, space="PSUM") as CP,
        ):
            # ---- persistent tiles ----
            kT_sb = P.tile([128, DC, S], BF16, tag="kT")
            qT_sb = P.tile([128, DC, QS], BF16, tag="qT")
            vplus = P.tile([128, SC128, 8 * 65], BF16, tag="vplus")
            vp4 = vplus.rearrange("p s (h c) -> p s h c", c=65)
            ebT = P.tile([128, SC128, QS], BF16, tag="ebT")
            ctxT_sb = P.tile([128, DC, QS], BF16, tag="ctxT")

            # ---- small input loads (sync queue) ----
            iota_col = P.tile([NC, 1], F32, tag="iota_col")
            nc.sync.dma_start(out=iota_col,
                              in_=iota64.rearrange("(p a) -> p a", a=1))
            cidkb = P.tile([NC, S], BF16, tag="cidkb")
            nc.sync.dma_start(out=cidkb, in_=bcast_ap(cidk, NC))
            cidqb = P.tile([NC, QS], BF16, tag="cidqb")
            nc.sync.dma_start(out=cidqb, in_=bcast_ap(cidq, NC))
            qposb = P.tile([128, QS], F16, tag="qposb")
            nc.sync.dma_start(out=qposb, in_=bcast_ap(qpos, 128))
            kposT_sb = P.tile([128, SC128], F32, tag="kposT")
            nc.sync.dma_start(out=kposT_sb, in_=kposT)
            b_sb = {}
            for nm, ap_ in (("bq", bq), ("bv", bv)):
                b_sb[nm] = P.tile([128, DC], F32, tag=nm, name=nm)
                nc.sync.dma_start(out=b_sb[nm],
                                  in_=ap_.rearrange("(c p) -> p c", p=128))
            bo_row = P.tile([1, D], F32, tag="bo_row")
            nc.sync.dma_start(out=bo_row, in_=bo.rearrange("(a s) -> a s", a=1))
            if with_masks:
                kpmT_sb = P.tile([128, SC128], F32, tag="kpmT")
                nc.sync.dma_start(out=kpmT_sb, in_=kpmT)

            # ---- big input loads (gpsimd queue; issue order ~ arrival) ----
            w_sb = {}

            def load_w(nm, ap_):
                w_sb[nm] = P.tile([128, DC, D], BF16, tag=nm, name=nm)
                nc.gpsimd.dma_start(
                    out=w_sb[nm], in_=ap_.rearrange("(c p) s -> p c s", p=128))

            load_w("wq", wqT)
            xqT_sb = P.tile([128, DC, QS], BF16, tag="xqT")
            nc.gpsimd.dma_start(
                out=xqT_sb, in_=xqT.rearrange("(c p) s -> p c s", p=128))
            load_w("wk", wkT)
            load_w("wv", wvT)
            xT_sb = P.tile([128, DC, S], BF16, tag="xT")

            def load_x_sc(sc):
                nc.gpsimd.dma_start(
                    out=xT_sb[:, :, sc * 512:(sc + 1) * 512],
                    in_=xT.rearrange("(c p) s -> p c s", p=128)[
                        :, :, sc * 512:(sc + 1) * 512])

            bias_in = {"ipa": [None] * 4, "asc": [None] * 4,
                       "msk": [None] * 4}

            def load_bias_g4(g4):
                specs = [("ipa", ipaT, IPA), ("asc", ascT, ASC)]
                if with_masks:
                    specs.append(("msk", mskT, EW))
                for nm, ap_, pool in specs:
                    t = pool.tile([128, 4, QS], BF16, tag="b" + nm, name=nm)
                    nc.gpsimd.dma_start(
                        out=t,
                        in_=ap_[g4 * 512:(g4 + 1) * 512, :].rearrange(
                            "(c p) s -> p c s", p=128))
                    bias_in[nm][g4] = t

            # biases first among the big streams: the eb pairs gate the
            # pf/attnV chain for EVERY head, while xT only gates the
            # progressive kT/v production consumed one block per h0 group
            load_bias_g4(0)
            load_bias_g4(1)
            load_x_sc(0)
            load_bias_g4(2)
            load_bias_g4(3)
            load_x_sc(1)
            load_x_sc(2)
            load_x_sc(3)
            load_w("wo", woT)
            wo_sb = w_sb["wo"]

            bv_bf = P.tile([128, DC], BF16, tag="bv_bf")
            nc.vector.tensor_copy(bv_bf, b_sb["bv"])
            cvec = P.tile([1, D], BF16, tag="cvec")
            ones_row = P.tile([1, 128], BF16, tag="ones_row")
            nc.vector.memset(ones_row, 1.0)
            nc.vector.memset(vp4[:, :, :, 64:65], 1.0)

            # ---- one-hot concept encodings (DVE, tiny) ----
            ohk = P.tile([NC, S], BF16, tag="ohk")
            nc.vector.tensor_scalar(
                out=ohk, in0=cidkb, scalar1=iota_col, scalar2=None,
                op0=AL.is_equal)
            ohq = P.tile([NC, QS], BF16, tag="ohq")
            nc.vector.tensor_scalar(
                out=ohq, in0=cidqb, scalar1=iota_col, scalar2=0.5,
                op0=AL.is_equal, op1=AL.mult)

            # ---- concept-score matmuls (PSUM tiles consumed promptly by the
            # combine now that biases are front-loaded in the DMA order) ----
            cs_tiles = []
            for p in range(8):
                cs = SPP.tile([128, 2, 512], F32, tag="sp", name="cs")
                for j in range(2):
                    kc = p * 2 + j
                    nc.tensor.matmul(
                        cs[:, j, :],
                        lhsT=ohk[:, kc * 128:(kc + 1) * 128],
                        rhs=ohq,
                        start=True, stop=True)
                cs_tiles.append(cs)

            # ---- combined bias -> exp, in [k, q] layout, per kc pair ----
            # xfall = cs*(qpos!=kpos) + ipa + asc (+ msk + kpm); eb = exp().
            def combine_pair(p):
                g4 = p // 2
                ipa_t = bias_in["ipa"][g4]
                asc_t = bias_in["asc"][g4]
                jj = (p % 2) * 2
                cs = cs_tiles[p]
                xf = XF.tile([128, 2, QS], BF16, tag="xf", name="xf")
                for j in range(2):
                    kc = p * 2 + j
                    # xf = (qpos != kpos) * cs  -- one fused op per chunk,
                    # reading cs straight from PSUM (frees the ring slot)
                    nc.vector.scalar_tensor_tensor(
                        out=xf[:, j, :], in0=qposb,
                        scalar=kposT_sb[:, kc:kc + 1], in1=cs[:, j, :],
                        op0=AL.not_equal, op1=AL.mult)
                t1 = XF.tile([128, 2, QS], BF16, tag="t1", name="t1")
                nc.vector.tensor_tensor(
                    out=t1, in0=xf, in1=ipa_t[:, jj:jj + 2, :], op=AL.add)
                dst = XF.tile([128, 2, QS], BF16, tag="t2", name="t2")
                nc.vector.tensor_tensor(
                    out=dst, in0=t1, in1=asc_t[:, jj:jj + 2, :], op=AL.add)
                if with_masks:
                    dst2 = XF.tile([128, 2, QS], BF16, tag="t3", name="t3")
                    nc.vector.tensor_tensor(
                        out=dst2, in0=dst, in1=bias_in["msk"][g4][:, jj:jj + 2, :],
                        op=AL.add)
                    kcol = EW.tile([128, 2, QS], BF16, tag="kp", name="kp")
                    for j in range(2):
                        kc = p * 2 + j
                        nc.vector.tensor_scalar(
                            out=kcol[:, j, :], in0=dst2[:, j, :],
                            scalar1=kpmT_sb[:, kc:kc + 1], scalar2=None,
                            op0=AL.add)
                    dst = kcol
                # exp on Act
                nc.scalar.activation(
                    out=ebT[:, p * 2:(p + 1) * 2, :], in_=dst, func=AF.Exp)

            for p in range(8):
                combine_pair(p)

            # ---- Q projection (+bq via Act identity) ----
            for ocp in range(2):
                ps = SPP.tile([128, 2, 512], F32, tag="sp", name="psq")
                for i in range(2):
                    oc = ocp * 2 + i
                    for dc in range(DC):
                        nc.tensor.matmul(
                            ps[:, i, :],
                            lhsT=w_sb["wq"][:, dc, oc * 128:(oc + 1) * 128],
                            rhs=xqT_sb[:, dc, :],
                            start=(dc == 0), stop=(dc == DC - 1))
                for i in range(2):
                    oc = ocp * 2 + i
                    nc.scalar.activation(
                        out=qT_sb[:, oc, :], in_=ps[:, i, :],
                        func=AF.Identity, bias=b_sb["bq"][:, oc:oc + 1])

            # ---- K projection (sc-major so scores can start early) ----
            for sc in range(SC512):
                for ocp in range(2):
                    ps = SPP.tile([128, 2, 512], F32, tag="sp", name="psk")
                    for i in range(2):
                        oc = ocp * 2 + i
                        for dc in range(DC):
                            nc.tensor.matmul(
                                ps[:, i, :],
                                lhsT=w_sb["wk"][:, dc,
                                                oc * 128:(oc + 1) * 128],
                                rhs=xT_sb[:, dc, sc * 512:(sc + 1) * 512],
                                start=(dc == 0), stop=(dc == DC - 1))
                    dst = kT_sb[:, ocp * 2:(ocp + 1) * 2,
                                sc * 512:(sc + 1) * 512]
                    if sc < 2:
                        nc.vector.tensor_copy(dst, ps)
                    else:
                        nc.scalar.copy(out=dst, in_=ps)

            # ---- V projection -> vplus (copies on gpsimd) ----
            for scp in range(8):
                ps = SPP.tile([128, 2, 512], F32, tag="sp", name="psv")
                for i in range(2):
                    sc = scp * 2 + i
                    for dc in range(DC):
                        nc.tensor.matmul(
                            ps[:, i, :],
                            lhsT=xT_sb[:, dc, sc * 128:(sc + 1) * 128],
                            rhs=w_sb["wv"][:, dc, :],
                            start=(dc == 0), stop=(dc == DC - 1))
                vdst = vp4[:, scp * 2:(scp + 1) * 2, :, 0:64]
                vsrc = ps.rearrange("p i (h c) -> p i h c", c=64)
                if scp % 2 == 0:
                    nc.vector.tensor_copy(vdst, vsrc)
                else:
                    nc.scalar.copy(out=vdst, in_=vsrc)

            # ---- attention main loop ----
            norm_state = {}

            def norm_step(step, hprev, cps_prev):
                ocp_, rbp = hprev // 2, (hprev % 2) * 64
                if step == 0:
                    rs_row = RW.tile([1, QS], F32, tag="rs_row",
                                     name="rs_row")
                    nc.vector.tensor_copy(rs_row, cps_prev[64:65, :])
                    rr = RW.tile([1, QS], F32, tag="rr", name="rr")
                    nc.vector.reciprocal_approx_fast(rr, rs_row)
                    norm_state["rr"] = rr
                elif step == 1:
                    rrb = RW.tile([64, QS], F32, tag="rrb", name="rrb")
                    nc.gpsimd.partition_broadcast(rrb, norm_state["rr"])
                    norm_state["rrb"] = rrb
                else:
                    nc.vector.tensor_tensor(
                        out=ctxT_sb[rbp:rbp + 64, ocp_, :],
                        in0=cps_prev[0:64, :], in1=norm_state["rrb"],
                        op=AL.mult)

            # attnV emission lags the scores/exp stream by ATTNV_LAG groups
            # so the in-order PE queue never stalls waiting for praw/pf of
            # the group it just produced (software pipelining).
            ATTNV_LAG = 6

            def emit_attnv(cps_t, h, g, pf):
                for j in range(2):
                    kc = g * 2 + j
                    nc.tensor.matmul(
                        cps_t,
                        lhsT=vplus[:, kc, h * 65:(h + 1) * 65],
                        rhs=pf[:, j, :],
                        start=(kc == 0), stop=(kc == SC128 - 1))

            pending = None
            attnv_q = []
            for h in range(H):
                oc, rb = h // 2, (h % 2) * 64
                cps_h = CP.tile([65, QS], F32, tag="ctx", name="ctx")
                for g in range(8):
                    ps = SPP.tile([128, 2, 512], F32, tag="sp", name="pss")
                    for j in range(2):
                        kc = g * 2 + j
                        nc.tensor.matmul(
                            ps[:, j, :],
                            lhsT=kT_sb[rb:rb + 64, oc,
                                       kc * 128:(kc + 1) * 128],
                            rhs=qT_sb[rb:rb + 64, oc, :],
                            start=True, stop=True)
                    praw = PRW.tile([128, 2, 512], BF16, tag="praw",
                                    name="praw")
                    nc.scalar.activation(out=praw, in_=ps, func=AF.Exp,
                                         scale=0.125)
                    pf = PW.tile([128, 2, 512], BF16, tag="pf", name="pf")
                    nc.vector.tensor_tensor(
                        out=pf, in0=praw,
                        in1=ebT[:, g * 2:(g + 1) * 2, :], op=AL.mult)
                    attnv_q.append((cps_h, h, g, pf))
                    if len(attnv_q) > ATTNV_LAG:
                        emit_attnv(*attnv_q.pop(0))
                    if pending is not None and 5 <= g <= 7:
                        norm_step(g - 5, *pending)
                        if g == 7:
                            pending = None
                pending = (h, cps_h)
            while attnv_q:
                emit_attnv(*attnv_q.pop(0))

            # cvec = Wo @ bv + bo  (rank-1 epilogue row)
            cps = SPP.tile([128, 2, 512], F32, tag="sp", name="cps")
            for dc in range(DC):
                nc.tensor.matmul(cps[0:1, 0, :], lhsT=bv_bf[:, dc:dc + 1],
                                 rhs=wo_sb[:, dc, :],
                                 start=(dc == 0), stop=(dc == DC - 1))
            nc.vector.tensor_tensor(out=cvec, in0=cps[0:1, 0, :], in1=bo_row,
                                    op=AL.add)

            for step in range(3):
                norm_step(step, *pending)

            # ---- output projection ----
            for m in range(QT):
                pom = SPP.tile([128, 2, 512], F32, tag="sp", name="pom")
                for dc in range(DC):
                    nc.tensor.matmul(
                        pom[:, 0, :],
                        lhsT=ctxT_sb[:, dc, m * 128:(m + 1) * 128],
                        rhs=wo_sb[:, dc, :],
                        start=(dc == 0), stop=False)
                nc.tensor.matmul(pom[:, 0, :], lhsT=ones_row, rhs=cvec,
                                 start=False, stop=True)
                o_t = OS.tile([128, 512], BF16, tag="o", name="o_t")
                nc.vector.tensor_copy(o_t, pom[:, 0, :])
                nc.sync.dma_start(out=out[m * 128:(m + 1) * 128, :],
                                  in_=o_t)

    nc.compile()
    return nc


def _prep_in_maps(inputs, with_masks):
    x = np.asarray(inputs["x"], np.float32)
    ipa = np.asarray(inputs["ipa_affinity_bias"], np.float32)
    asc = np.asarray(inputs["assoc_bias"], np.float32)
    msk = np.asarray(inputs["attention_mask"], np.float32)
    cid = np.asarray(inputs["concept_ids"])
    kpm = np.asarray(inputs["key_padding_mask"])

    wT = {nm: np.ascontiguousarray(
        np.asarray(inputs[nm], np.float32).T.astype(BF))
        for nm in ("Wq", "Wk", "Wv", "Wo")}
    bias = {nm: np.asarray(inputs[nm], np.float32)
            for nm in ("bq", "bv", "bo")}

    xT = [np.ascontiguousarray(x[b].T.astype(BF)) for b in range(B)]
    ipaT = [[np.ascontiguousarray(
        ipa[b, q0:q0 + QS].T.astype(BF)) for q0 in range(0, S, QS)]
        for b in range(B)]
    ascT = [[np.ascontiguousarray(
        asc[b, q0:q0 + QS].T.astype(BF)) for q0 in range(0, S, QS)]
        for b in range(B)]
    cidq_f = np.where(cid >= 0, cid, -1).astype(BF)
    cidk_f = np.where(cid >= 0, cid, -2).astype(BF)
    kpm_add = np.where(kpm, np.float32(-1e30), np.float32(0.0))
    kpos = np.arange(S, dtype=np.float32)
    iota64 = np.arange(NC, dtype=np.float32)

    in_maps = []
    for c in range(N_CORES):
        b, qi = c // 4, c % 4
        q0 = qi * QS
        m = {
            "xT": xT[b],
            "xqT": np.ascontiguousarray(xT[b][:, q0:q0 + QS]),
            "wqT": wT["Wq"], "wkT": wT["Wk"],
            "wvT": wT["Wv"], "woT": wT["Wo"],
            "bq": bias["bq"], "bv": bias["bv"], "bo": bias["bo"],
            "ipaT": ipaT[b][qi],
            "ascT": ascT[b][qi],
            "cidq": np.ascontiguousarray(cidq_f[b, q0:q0 + QS]),
            "cidk": np.ascontiguousarray(cidk_f[b]),
            "iota64": iota64,
            "qpos": (q0 + np.arange(QS)).astype(np.float16),
            "kposT": np.ascontiguousarray(kpos.reshape(SC128, 128).T),
        }
        if with_masks:
            m["mskT"] = np.ascontiguousarray(
                msk[q0:q0 + QS].T.astype(BF))
            m["kpmT"] = np.ascontiguousarray(
                kpm_add[b].reshape(SC128, 128).T)
        in_maps.append(m)
    return in_maps


def run(inputs, trace=False):
    msk = np.asarray(inputs["attention_mask"])
    kpm = np.asarray(inputs["key_padding_mask"])
    with_masks = bool(msk.any() or kpm.any())
    if with_masks not in _COMPILED:
        _COMPILED[with_masks] = _build(with_masks)
    nc = _COMPILED[with_masks]
    in_maps = _prep_in_maps(inputs, with_masks)
    kw = {}
    if trace:
        kw = dict(trace=True, trace_cores=list(range(N_CORES)))
    res = bass_utils.run_bass_kernel_spmd(
        nc, in_maps, core_ids=list(range(N_CORES)), **kw)
    out = np.empty((B, S, D), np.float32)
    for c in range(N_CORES):
        b, q0 = c // 4, (c % 4) * QS
        out[b, q0:q0 + QS] = np.asarray(res.results[c]["out"],
                                        dtype=np.float32)
    return out, res


def kernel(**inputs) -> np.ndarray:
    out, _ = run(inputs)
    return out
